# revision 7
# baseline (speedup 1.0000x reference)
"""EnsembleRBF Trainium2 kernel: out[m,n,d] = sum_c exp(-||x_n - c_c||^2) * sigma^2 * w[m,c,d].

v5 design (ACT-bound target ~25us/core):
  Data-parallel along N across 8 cores (12800 padded rows/core), n = p*100 + b
  (p = SBUF partition, b = 128-col block 0..99).
  Host precomputes fp8(E4M3) feature decompositions of both matmul operands:
  24 virtual contraction rows as 12 partition rows x 2 (DoubleRow pairs):
    d2[c, n] = sum_k' fx_k'(x_n) * fc_k'(c) = |x-c|^2  (+/- ~4e-3)
  with 3-term fp8 splits of x0, x1, -2c0, -2c1 (8 product pairs per dim) and
  4-term splits of |x|^2, |c|^2 against ones.
  Per-core loop over 25 chunks of 4 blocks:
    MM1 (PE): 2 fp8 DoubleRow matmuls (FD=512 out cols each, 0.5 cyc/row
      even at the HAM-throttled 1.2 GHz PE clock) -> d2 [128, 1024] fp32 PSUM
      (2 banks, bufs=3 -> 6 banks: MM1 runs 2 ACT-periods ahead)
    ACT: one exp(-d2) op per chunk, FD=1024 (25 ops ~= 25us = the bottleneck;
      table load hoisted to t=0 via dummy exp)
    MM2 (PE): per block, rbf[c, nblock] as FWL fp16 stationary + FD=16 moving
      wr -> po[j, 16i+(m,d)] PSUM (bufs=2 -> 2 banks; ~27ns/pair)
    DVE: copy po -> stage (m,b,d)-major fp32
  MM2 emission delayed 2 chunks so MM1(ch) precedes MM2(ch-2) in PE program
  order. 4 single-DMA output waves (all 5 models in one 4D-AP DMA each).
"""
import numpy as np
import ml_dtypes

import concourse.bass as bass
import concourse.tile as tile
from concourse import bacc, mybir
from concourse.bass_utils import run_bass_kernel_spmd

N, C, D, M = 100000, 256, 2, 5
SIGMA2 = 0.0625
NCORES = 8
NCP = 12800          # padded rows per core
NBLK = NCP // 128    # 100 blocks, n = p*100 + b
f32 = mybir.dt.float32
f16 = mybir.dt.float16
f8 = mybir.dt.float8e4
E4 = ml_dtypes.float8_e4m3

_CACHE = {}

CHUNK = 4            # blocks per chunk
NCHUNK = NBLK // CHUNK   # 25
KF = 12              # fp8 feature partition rows (24 virtual rows as 12 x 2)


def _build():
    nc = bacc.Bacc("TRN2", target_bir_lowering=False, debug=False, num_devices=NCORES)
    rx_ap = nc.dram_tensor("rx", [KF, 2, NCP], f8, kind="ExternalInput").ap()
    augc_ap = nc.dram_tensor("augc", [KF, 512], f8, kind="ExternalInput").ap()
    wr_ap = nc.dram_tensor("wr", [128, 32], f16, kind="ExternalInput").ap()
    out_ap = nc.dram_tensor("out", [M, NCP, 2], f32, kind="ExternalOutput").ap()

    Exp = mybir.ActivationFunctionType.Exp
    DR = mybir.MatmulPerfMode.DoubleRow

    with tile.TileContext(nc) as tc:
        with (
            tc.tile_pool(name="consts", bufs=1) as consts,
            tc.tile_pool(name="d2p", bufs=3, space="PSUM") as d2_pool,
            tc.tile_pool(name="pop", bufs=2, space="PSUM") as po_pool,
        ):
            augc = consts.tile([KF, 512], f8)
            wr = consts.tile([128, 32], f16)
            rhs_b = consts.tile([KF, 2 * NCP], f8)
            rbf = consts.tile([128, 256 * NBLK], f16)
            stage = consts.tile([128, M * NBLK * 2], f32)
            dum_i = consts.tile([128, 1], f32)
            dum_o = consts.tile([128, 1], f16)

            # hoist ACT table load to t~0
            nc.vector.memset(dum_i[:], 0.0)
            nc.scalar.activation(dum_o[:], dum_i[:], Exp, scale=-1.0)

            rb3 = rhs_b[:].rearrange("p (e j) -> p e j", e=2)
            rx3 = rx_ap  # [KF, 2, NCP]
            # head chunk first so MM1_0 can start asap, then the rest
            nc.sync.dma_start(rb3[:, :, 0:512], rx3[:, :, 0:512])
            nc.sync.dma_start(augc[:], augc_ap[:])
            nc.sync.dma_start(rb3[:, :, 512:6400], rx3[:, :, 512:6400])
            nc.sync.dma_start(wr[:], wr_ap[:])
            nc.sync.dma_start(rb3[:, :, 6400:], rx3[:, :, 6400:])

            augc4 = augc[:].rearrange("p (cc e c) -> p cc e c", cc=2, c=128)
            stv = stage[:].rearrange("p (m b d) -> p m b d", m=M, d=2)

            def mm2(ch):
                b0 = CHUNK * ch
                fd = CHUNK * 128
                rb = 256 * b0
                po = po_pool.tile([128, 16 * CHUNK], f32, tag="po")
                for i in range(CHUNK):
                    nc.tensor.matmul(
                        po[:, 16 * i : 16 * i + 16],
                        rbf[:, rb + 128 * i : rb + 128 * i + 128],
                        wr[:, 0:16],
                        start=True,
                        stop=False,
                    )
                    nc.tensor.matmul(
                        po[:, 16 * i : 16 * i + 16],
                        rbf[:, rb + fd + 128 * i : rb + fd + 128 * i + 128],
                        wr[:, 16:32],
                        start=False,
                        stop=True,
                    )
                pov = po[:].rearrange("p (i m d) -> p m i d", m=8, d=2)
                nc.vector.tensor_copy(
                    stv[:, :, b0 : b0 + CHUNK, :], pov[:, 0:M, 0:CHUNK, :]
                )

            def wave(blo, bhi):
                # one DMA for all 5 models: element order [p][m][b][d] on both
                dst = out_ap.rearrange("m (p b) d -> p m b d", p=128)[
                    :, :, blo:bhi, :
                ]
                nc.sync.dma_start(dst, stv[:, :, blo:bhi, :])

            for ch in range(NCHUNK):
                b0 = CHUNK * ch
                fd = CHUNK * 128
                d2 = d2_pool.tile([128, 2 * CHUNK * 128], f32, tag="d2")
                for cc in range(2):
                    nc.tensor.matmul(
                        d2[:, cc * fd : (cc + 1) * fd],
                        augc4[:, cc, :, :],
                        rb3[:, :, 128 * b0 : 128 * b0 + fd],
                        start=True,
                        stop=True,
                        perf_mode=DR,
                    )
                nc.scalar.activation(
                    rbf[:, 256 * b0 : 256 * b0 + 2 * fd], d2[:, 0 : 2 * fd],
                    Exp, scale=-1.0,
                )
                if ch >= 2:
                    mm2(ch - 2)
                if ch == 8:
                    wave(0, 24)     # stages 0..5 done
                if ch == 14:
                    wave(24, 48)    # stages 6..11 done
                if ch == 20:
                    wave(48, 72)    # stages 12..17 done
            mm2(NCHUNK - 2)
            mm2(NCHUNK - 1)
            wave(72, NBLK)

    nc.compile()
    return nc


def _q8(a):
    return np.asarray(a, np.float32).astype(E4)


def _split8(a, k):
    terms, r = [], np.asarray(a, np.float32).copy()
    for _ in range(k):
        t = _q8(r)
        terms.append(t)
        r = r - t.astype(np.float32)
    return terms


_PAIRS = [(0, 0), (0, 1), (1, 0), (0, 2), (2, 0), (1, 1), (1, 2), (2, 1)]


def _host_prep(x, centers, weights):
    x = np.ascontiguousarray(np.asarray(x, dtype=np.float32))
    centers = np.asarray(centers, dtype=np.float32)
    weights = np.asarray(weights, dtype=np.float32)

    xp = np.zeros((NCORES * NCP, 2), np.float32)
    xp[:N] = x

    # 24 (fx, fc) fp8 product pairs: sum_k' fx_k'(x) * fc_k'(c) = |x-c|^2
    fxs, fcs = [], []
    for dim in range(2):
        xt = _split8(xp[:, dim], 3)
        ct = _split8(-2.0 * centers[:, dim], 3)
        for a, b in _PAIRS:
            fxs.append(xt[a])
            fcs.append(ct[b])
    x2 = np.sum(xp * xp, axis=1, dtype=np.float32)
    c2 = np.sum(centers * centers, axis=1, dtype=np.float32)
    onesN = np.ones(NCORES * NCP, E4)
    onesC = np.ones(C, E4)
    for t in _split8(x2, 4):
        fxs.append(t)
        fcs.append(onesC)
    for t in _split8(c2, 4):
        fxs.append(onesN)
        fcs.append(t)

    fx = np.stack(fxs)                                  # [24, NCORES*NCP]
    fc = np.stack(fcs)                                  # [24, C]

    # rx[core][k, e, 128*b + j] = fx[2k+e, core_base + j*100 + b]
    fv = fx.reshape(KF, 2, NCORES, 128, NBLK)           # [k, e, core, j, b]
    rx = np.ascontiguousarray(fv.transpose(2, 0, 1, 4, 3)).reshape(
        NCORES, KF, 2, NCP
    )

    # augc[k, (cc, e, c_local)] = fc[2k+e, cc*128 + c_local]
    fcv = fc.reshape(KF, 2, 2, 128)                     # [k, e, cc, c]
    augc = np.ascontiguousarray(fcv.transpose(0, 2, 1, 3)).reshape(KF, 512)

    wmd = (weights * SIGMA2).transpose(1, 0, 2).reshape(C, 10).astype(np.float16)
    wr = np.zeros((128, 32), np.float16)
    wr[:, 0:10] = wmd[:128]
    wr[:, 16:26] = wmd[128:]
    return rx, augc, wr


def kernel(x, centers, weights):
    if "nc" not in _CACHE:
        _CACHE["nc"] = _build()
    nc = _CACHE["nc"]
    rx, augc, wr = _host_prep(x, centers, weights)
    in_maps = [{"rx": rx[i], "augc": augc, "wr": wr} for i in range(NCORES)]
    res = run_bass_kernel_spmd(nc, in_maps, list(range(NCORES)))
    outs = np.concatenate([res.results[i]["out"] for i in range(NCORES)], axis=1)
    return np.ascontiguousarray(outs[:, :N, :])


# revision 8
# speedup vs baseline: 1.1289x; 1.1289x over previous
"""EnsembleRBF Trainium2 kernel: out[m,n,d] = sum_c exp(-||x_n - c_c||^2) * sigma^2 * w[m,c,d].

v6 design (ACT-bound target ~25us/core):
  Data-parallel along N across 8 cores (12800 padded rows/core), n = p*100 + b
  (p = SBUF partition, b = 128-col block 0..99).
  Host precomputes fp16 hi/lo feature splits of both matmul operands:
    rhs_b[k, 128b + j] = feat_k(x[j*100 + b])   (10 rows x 12800, replicated
      at partition bases 0 and 64 for the two concurrent row-tiles)
    augc[k, c]: rows 0:10 = center features for c 0..127, rows 64:74 = for
      c 128..255
  Per-core loop over 25 chunks of 4 blocks:
    MM1 (PE): TWO CONCURRENT row-tiled matmuls (K=10 at row groups 0 and 64;
      both stream FD=512 simultaneously through separate XBUSes) -> d2
      [128, 1024] fp32 PSUM. One row-group per PSUM bank (cc0 -> bank 0,
      cc1 -> bank 1, invariant over all chunks). bufs=3 -> 6 banks: MM1 runs
      2 ACT-periods ahead of the exp.
    ACT: one exp(-d2) op per chunk, FD=1024 (25 ops ~ 25us = the bottleneck;
      table load hoisted to t=0 via dummy exp).
    MM2 (PE): per block, rbf[c, nblock] as FWL fp16 stationary + FD=16 moving
      wr -> po[j, 16i+(m,d)] PSUM (bufs=2 -> 2 banks; ~27ns/pair).
    DVE: copy po -> stage (m,b,d)-major fp32.
  MM2 emission delayed 2 chunks so MM1(ch) precedes MM2(ch-2) in PE program
  order. 4 single-DMA output waves (all 5 models in one 4D-AP DMA each).
"""
import numpy as np

import concourse.bass as bass
import concourse.tile as tile
from concourse import bacc, mybir
from concourse.bass_utils import run_bass_kernel_spmd

N, C, D, M = 100000, 256, 2, 5
SIGMA2 = 0.0625
NCORES = 8
NCP = 12800          # padded rows per core
NBLK = NCP // 128    # 100 blocks, n = p*100 + b
f32 = mybir.dt.float32
f16 = mybir.dt.float16

_CACHE = {}

CHUNK = 4            # blocks per chunk
NCHUNK = NBLK // CHUNK   # 25


def _build():
    nc = bacc.Bacc("TRN2", target_bir_lowering=False, debug=False, num_devices=NCORES)
    rx_ap = nc.dram_tensor("rx", [10, NCP], f16, kind="ExternalInput").ap()
    augc_ap = nc.dram_tensor("augc", [128, 256], f16, kind="ExternalInput").ap()
    wr_ap = nc.dram_tensor("wr", [128, 32], f16, kind="ExternalInput").ap()
    out_ap = nc.dram_tensor("out", [M, NCP, 2], f32, kind="ExternalOutput").ap()

    Exp = mybir.ActivationFunctionType.Exp

    with tile.TileContext(nc) as tc:
        with (
            tc.tile_pool(name="consts", bufs=1) as consts,
            tc.tile_pool(name="d2p", bufs=3, space="PSUM") as d2_pool,
            tc.tile_pool(name="pop", bufs=2, space="PSUM") as po_pool,
        ):
            augc = consts.tile([128, 256], f16)
            wr = consts.tile([128, 32], f16)
            rhs_b = consts.tile([128, NCP], f16)
            rbf = consts.tile([128, 256 * NBLK], f16)
            stage = consts.tile([128, M * NBLK * 2], f32)
            dum_i = consts.tile([128, 1], f32)
            dum_o = consts.tile([128, 1], f16)

            # hoist ACT table load to t~0
            nc.vector.memset(dum_i[:], 0.0)
            nc.scalar.activation(dum_o[:], dum_i[:], Exp, scale=-1.0)

            # head chunk first (both row-tile replicas) so MM1_0 starts asap
            nc.sync.dma_start(rhs_b[0:10, 0:512], rx_ap[:, 0:512])
            nc.sync.dma_start(rhs_b[64:74, 0:512], rx_ap[:, 0:512])
            nc.sync.dma_start(augc[:], augc_ap[:])
            nc.sync.dma_start(rhs_b[0:10, 512:6400], rx_ap[:, 512:6400])
            nc.sync.dma_start(rhs_b[64:74, 512:6400], rx_ap[:, 512:6400])
            nc.sync.dma_start(wr[:], wr_ap[:])
            nc.sync.dma_start(rhs_b[0:10, 6400:], rx_ap[:, 6400:])
            nc.sync.dma_start(rhs_b[64:74, 6400:], rx_ap[:, 6400:])

            stv = stage[:].rearrange("p (m b d) -> p m b d", m=M, d=2)

            def mm2(ch):
                b0 = CHUNK * ch
                fd = CHUNK * 128
                rb = 256 * b0
                po = po_pool.tile([128, 16 * CHUNK], f32, tag="po")
                for i in range(CHUNK):
                    nc.tensor.matmul(
                        po[:, 16 * i : 16 * i + 16],
                        rbf[:, rb + 128 * i : rb + 128 * i + 128],
                        wr[:, 0:16],
                        start=True,
                        stop=False,
                    )
                    nc.tensor.matmul(
                        po[:, 16 * i : 16 * i + 16],
                        rbf[:, rb + fd + 128 * i : rb + fd + 128 * i + 128],
                        wr[:, 16:32],
                        start=False,
                        stop=True,
                    )
                pov = po[:].rearrange("p (i m d) -> p m i d", m=8, d=2)
                nc.vector.tensor_copy(
                    stv[:, :, b0 : b0 + CHUNK, :], pov[:, 0:M, 0:CHUNK, :]
                )

            def wave(blo, bhi):
                # one DMA for all 5 models: element order [p][m][b][d] on both
                dst = out_ap.rearrange("m (p b) d -> p m b d", p=128)[
                    :, :, blo:bhi, :
                ]
                nc.sync.dma_start(dst, stv[:, :, blo:bhi, :])

            for ch in range(NCHUNK):
                b0 = CHUNK * ch
                fd = CHUNK * 128
                d2 = d2_pool.tile([128, 2 * CHUNK * 128], f32, tag="d2")
                # two concurrent row-tiled matmuls: row group 0 -> c 0..127
                # (bank 0), row group 64 -> c 128..255 (bank 1)
                nc.tensor.matmul(
                    d2[:, 0:fd],
                    augc[0:10, 0:128],
                    rhs_b[0:10, 128 * b0 : 128 * b0 + fd],
                    start=True,
                    stop=True,
                )
                nc.tensor.matmul(
                    d2[:, fd : 2 * fd],
                    augc[64:74, 128:256],
                    rhs_b[64:74, 128 * b0 : 128 * b0 + fd],
                    start=True,
                    stop=True,
                )
                nc.scalar.activation(
                    rbf[:, 256 * b0 : 256 * b0 + 2 * fd], d2[:, 0 : 2 * fd],
                    Exp, scale=-1.0,
                )
                if ch >= 2:
                    mm2(ch - 2)
                if ch == 8:
                    wave(0, 24)     # stages 0..5 done
                if ch == 14:
                    wave(24, 48)    # stages 6..11 done
                if ch == 20:
                    wave(48, 72)    # stages 12..17 done
            mm2(NCHUNK - 2)
            mm2(NCHUNK - 1)
            wave(72, NBLK)

    nc.compile()
    return nc


def _host_prep(x, centers, weights):
    x = np.ascontiguousarray(np.asarray(x, dtype=np.float32))
    centers = np.asarray(centers, dtype=np.float32)
    weights = np.asarray(weights, dtype=np.float32)

    xp = np.zeros((NCORES * NCP, 2), np.float32)
    xp[:N] = x

    # x-side features, hi/lo fp16 split: [xh0, xh0, xl0, xh1, xh1, xl1,
    # x2h, x2l, 1, 1] per point
    xh = xp.astype(np.float16)
    xl = (xp - xh.astype(np.float32)).astype(np.float16)
    x2 = np.sum(xp * xp, axis=1, dtype=np.float32)
    x2h = x2.astype(np.float16)
    x2l = (x2 - x2h.astype(np.float32)).astype(np.float16)
    ones = np.ones(NCORES * NCP, np.float16)
    feats = np.stack([
        xh[:, 0], xh[:, 0], xl[:, 0], xh[:, 1], xh[:, 1], xl[:, 1],
        x2h, x2l, ones, ones,
    ])  # [10, NCORES*NCP]

    # rx[core][k, 128*b + j] = feats[k, core_base + j*100 + b]
    fv = feats.reshape(10, NCORES, 128, NBLK)          # [k, core, j(p), b]
    rx = np.ascontiguousarray(fv.transpose(1, 0, 3, 2)).reshape(
        NCORES, 10, NCP
    )  # [core, k, (b, j)]

    ch = centers.astype(np.float16)
    cl = (centers - ch.astype(np.float32)).astype(np.float16)
    c2 = np.sum(centers * centers, axis=1, dtype=np.float32)
    c2h = c2.astype(np.float16)
    c2l = (c2 - c2h.astype(np.float32)).astype(np.float16)
    onesC = np.ones(C, np.float16)

    cf = np.stack([
        -2 * ch[:, 0], -2 * cl[:, 0], -2 * ch[:, 0],
        -2 * ch[:, 1], -2 * cl[:, 1], -2 * ch[:, 1],
        onesC, onesC, c2h, c2l,
    ])  # [10, 256]
    augc = np.zeros((128, 256), np.float16)
    augc[0:10, 0:128] = cf[:, 0:128]
    augc[64:74, 128:256] = cf[:, 128:256]

    wmd = (weights * SIGMA2).transpose(1, 0, 2).reshape(C, 10).astype(np.float16)
    wr = np.zeros((128, 32), np.float16)
    wr[:, 0:10] = wmd[:128]
    wr[:, 16:26] = wmd[128:]
    return rx, augc, wr


def kernel(x, centers, weights):
    if "nc" not in _CACHE:
        _CACHE["nc"] = _build()
    nc = _CACHE["nc"]
    rx, augc, wr = _host_prep(x, centers, weights)
    in_maps = [{"rx": rx[i], "augc": augc, "wr": wr} for i in range(NCORES)]
    res = run_bass_kernel_spmd(nc, in_maps, list(range(NCORES)))
    outs = np.concatenate([res.results[i]["out"] for i in range(NCORES)], axis=1)
    return np.ascontiguousarray(outs[:, :N, :])


# revision 10
# speedup vs baseline: 1.1715x; 1.0377x over previous
"""EnsembleRBF Trainium2 kernel: out[m,n,d] = sum_c exp(-||x_n - c_c||^2) * sigma^2 * w[m,c,d].

v6 design (ACT-bound target ~25us/core):
  Data-parallel along N across 8 cores (12800 padded rows/core), n = p*100 + b
  (p = SBUF partition, b = 128-col block 0..99).
  Host precomputes fp16 hi/lo feature splits of both matmul operands:
    rhs_b[k, 128b + j] = feat_k(x[j*100 + b])   (10 rows x 12800, replicated
      at partition bases 0 and 64 for the two concurrent row-tiles)
    augc[k, c]: rows 0:10 = center features for c 0..127, rows 64:74 = for
      c 128..255
  Per-core loop over 25 chunks of 4 blocks:
    MM1 (PE): TWO CONCURRENT row-tiled matmuls (K=10 at row groups 0 and 64;
      both stream FD=512 simultaneously through separate XBUSes) -> d2
      [128, 1024] fp32 PSUM. One row-group per PSUM bank (cc0 -> bank 0,
      cc1 -> bank 1, invariant over all chunks). bufs=3 -> 6 banks: MM1 runs
      2 ACT-periods ahead of the exp.
    ACT: one exp(-d2) op per chunk, FD=1024 (25 ops ~ 25us = the bottleneck;
      table load hoisted to t=0 via dummy exp).
    MM2 (PE): per block, rbf[c, nblock] as FWL fp16 stationary + FD=16 moving
      wr -> po[j, 16i+(m,d)] PSUM (bufs=2 -> 2 banks; ~27ns/pair).
    DVE: copy po -> stage (m,b,d)-major fp32.
  MM2 emission delayed 2 chunks so MM1(ch) precedes MM2(ch-2) in PE program
  order. 4 single-DMA output waves (all 5 models in one 4D-AP DMA each).
"""
import numpy as np

import concourse.bass as bass
import concourse.tile as tile
from concourse import bacc, mybir
from concourse.bass_utils import run_bass_kernel_spmd

N, C, D, M = 100000, 256, 2, 5
SIGMA2 = 0.0625
NCORES = 8
NCP = 12800          # padded rows per core
NBLK = NCP // 128    # 100 blocks, n = p*100 + b
f32 = mybir.dt.float32
f16 = mybir.dt.float16

_CACHE = {}

CHUNK = 4            # blocks per chunk
NCHUNK = NBLK // CHUNK   # 25


def _build():
    nc = bacc.Bacc("TRN2", target_bir_lowering=False, debug=False, num_devices=NCORES)
    rx_ap = nc.dram_tensor("rx", [10, NCP], f16, kind="ExternalInput").ap()
    augc_ap = nc.dram_tensor("augc", [128, 256], f16, kind="ExternalInput").ap()
    wr_ap = nc.dram_tensor("wr", [128, 32], f16, kind="ExternalInput").ap()
    out_ap = nc.dram_tensor("out", [M, NCP, 2], f32, kind="ExternalOutput").ap()

    Exp = mybir.ActivationFunctionType.Exp

    with tile.TileContext(nc) as tc:
        with (
            tc.tile_pool(name="consts", bufs=1) as consts,
            tc.tile_pool(name="d2p", bufs=3, space="PSUM") as d2_pool,
            tc.tile_pool(name="pop", bufs=2, space="PSUM") as po_pool,
        ):
            augc = consts.tile([128, 256], f16)
            wr = consts.tile([128, 32], f16)
            rhs_b = consts.tile([128, NCP], f16)
            rbf = consts.tile([128, 256 * NBLK], f16)
            stage = consts.tile([128, M * NBLK * 2], f32)
            dum_i = consts.tile([128, 1], f32)
            dum_o = consts.tile([128, 1], f16)

            # hoist ACT table load to t~0
            nc.vector.memset(dum_i[:], 0.0)
            nc.scalar.activation(dum_o[:], dum_i[:], Exp, scale=-1.0)

            # head chunk first (both row-tile replicas) so MM1_0 starts asap;
            # replicas split across the sync and scalar HWDGE queues (the ACT
            # engine is idle until ACT_0, so its DMA triggers are free here)
            nc.sync.dma_start(augc[:], augc_ap[:])
            nc.scalar.dma_start(rhs_b[64:74, 0:512], rx_ap[:, 0:512])
            nc.sync.dma_start(rhs_b[0:10, 0:512], rx_ap[:, 0:512])
            nc.scalar.dma_start(wr[:], wr_ap[:])
            nc.sync.dma_start(rhs_b[0:10, 512:2560], rx_ap[:, 512:2560])
            nc.scalar.dma_start(rhs_b[64:74, 512:2560], rx_ap[:, 512:2560])
            nc.sync.dma_start(rhs_b[0:10, 2560:], rx_ap[:, 2560:])
            nc.scalar.dma_start(rhs_b[64:74, 2560:], rx_ap[:, 2560:])

            stv = stage[:].rearrange("p (m b d) -> p m b d", m=M, d=2)

            def mm2(ch):
                b0 = CHUNK * ch
                fd = CHUNK * 128
                rb = 256 * b0
                po = po_pool.tile([128, 16 * CHUNK], f32, tag="po")
                for i in range(CHUNK):
                    nc.tensor.matmul(
                        po[:, 16 * i : 16 * i + 16],
                        rbf[:, rb + 128 * i : rb + 128 * i + 128],
                        wr[:, 0:16],
                        start=True,
                        stop=False,
                    )
                    nc.tensor.matmul(
                        po[:, 16 * i : 16 * i + 16],
                        rbf[:, rb + fd + 128 * i : rb + fd + 128 * i + 128],
                        wr[:, 16:32],
                        start=False,
                        stop=True,
                    )
                pov = po[:].rearrange("p (i m d) -> p m i d", m=8, d=2)
                nc.vector.tensor_copy(
                    stv[:, :, b0 : b0 + CHUNK, :], pov[:, 0:M, 0:CHUNK, :]
                )

            def wave(blo, bhi):
                # one DMA for all 5 models: element order [p][m][b][d] on both
                dst = out_ap.rearrange("m (p b) d -> p m b d", p=128)[
                    :, :, blo:bhi, :
                ]
                nc.sync.dma_start(dst, stv[:, :, blo:bhi, :])

            for ch in range(NCHUNK):
                b0 = CHUNK * ch
                fd = CHUNK * 128
                d2 = d2_pool.tile([128, 2 * CHUNK * 128], f32, tag="d2")
                # two concurrent row-tiled matmuls: row group 0 -> c 0..127
                # (bank 0), row group 64 -> c 128..255 (bank 1)
                nc.tensor.matmul(
                    d2[:, 0:fd],
                    augc[0:10, 0:128],
                    rhs_b[0:10, 128 * b0 : 128 * b0 + fd],
                    start=True,
                    stop=True,
                )
                nc.tensor.matmul(
                    d2[:, fd : 2 * fd],
                    augc[64:74, 128:256],
                    rhs_b[64:74, 128 * b0 : 128 * b0 + fd],
                    start=True,
                    stop=True,
                )
                nc.scalar.activation(
                    rbf[:, 256 * b0 : 256 * b0 + 2 * fd], d2[:, 0 : 2 * fd],
                    Exp, scale=-1.0,
                )
                if ch >= 2:
                    mm2(ch - 2)
                if ch == 8:
                    wave(0, 24)     # stages 0..5 done
                if ch == 14:
                    wave(24, 48)    # stages 6..11 done
                if ch == 20:
                    wave(48, 72)    # stages 12..17 done
            mm2(NCHUNK - 2)
            wave(72, 96)
            mm2(NCHUNK - 1)
            wave(96, NBLK)

    nc.compile()
    return nc


def _host_prep(x, centers, weights):
    x = np.ascontiguousarray(np.asarray(x, dtype=np.float32))
    centers = np.asarray(centers, dtype=np.float32)
    weights = np.asarray(weights, dtype=np.float32)

    xp = np.zeros((NCORES * NCP, 2), np.float32)
    xp[:N] = x

    # x-side features, hi/lo fp16 split: [xh0, xh0, xl0, xh1, xh1, xl1,
    # x2h, x2l, 1, 1] per point
    xh = xp.astype(np.float16)
    xl = (xp - xh.astype(np.float32)).astype(np.float16)
    x2 = np.sum(xp * xp, axis=1, dtype=np.float32)
    x2h = x2.astype(np.float16)
    x2l = (x2 - x2h.astype(np.float32)).astype(np.float16)
    ones = np.ones(NCORES * NCP, np.float16)
    feats = np.stack([
        xh[:, 0], xh[:, 0], xl[:, 0], xh[:, 1], xh[:, 1], xl[:, 1],
        x2h, x2l, ones, ones,
    ])  # [10, NCORES*NCP]

    # rx[core][k, 128*b + j] = feats[k, core_base + j*100 + b]
    fv = feats.reshape(10, NCORES, 128, NBLK)          # [k, core, j(p), b]
    rx = np.ascontiguousarray(fv.transpose(1, 0, 3, 2)).reshape(
        NCORES, 10, NCP
    )  # [core, k, (b, j)]

    ch = centers.astype(np.float16)
    cl = (centers - ch.astype(np.float32)).astype(np.float16)
    c2 = np.sum(centers * centers, axis=1, dtype=np.float32)
    c2h = c2.astype(np.float16)
    c2l = (c2 - c2h.astype(np.float32)).astype(np.float16)
    onesC = np.ones(C, np.float16)

    cf = np.stack([
        -2 * ch[:, 0], -2 * cl[:, 0], -2 * ch[:, 0],
        -2 * ch[:, 1], -2 * cl[:, 1], -2 * ch[:, 1],
        onesC, onesC, c2h, c2l,
    ])  # [10, 256]
    augc = np.zeros((128, 256), np.float16)
    augc[0:10, 0:128] = cf[:, 0:128]
    augc[64:74, 128:256] = cf[:, 128:256]

    wmd = (weights * SIGMA2).transpose(1, 0, 2).reshape(C, 10).astype(np.float16)
    wr = np.zeros((128, 32), np.float16)
    wr[:, 0:10] = wmd[:128]
    wr[:, 16:26] = wmd[128:]
    return rx, augc, wr


def kernel(x, centers, weights):
    if "nc" not in _CACHE:
        _CACHE["nc"] = _build()
    nc = _CACHE["nc"]
    rx, augc, wr = _host_prep(x, centers, weights)
    in_maps = [{"rx": rx[i], "augc": augc, "wr": wr} for i in range(NCORES)]
    res = run_bass_kernel_spmd(nc, in_maps, list(range(NCORES)))
    outs = np.concatenate([res.results[i]["out"] for i in range(NCORES)], axis=1)
    return np.ascontiguousarray(outs[:, :N, :])
